# revision 27
# baseline (speedup 1.0000x reference)
"""Trainium2 Bass kernel for nn_PoolHiddenNet (gnn_message_passing).

Reference computation (uniform contiguous groups of P=16):
    pos = in_xy[-1]                       # (B, 2)
    rel[g,i,j] = pos[g,j] - pos[g,i]
    emb = rel @ W_emb + b_emb             # (G,P,P,E)
    x   = concat([emb, h[g,j]], -1)
    x1  = relu(x @ W1 + b1)               # (G,P,P,H)
    x2  = relu(x1 @ W2 + b2)              # (G,P,P,BOT)
    out = max over j -> (B, BOT)

Algebraic restructuring:
    x1[g,i,j] = relu(u[g,j] - v[g,i])
       u[g,r]  = pos[g,r] @ (W_emb @ W1[:E]) + h[g,r] @ W1[E:] + (b_emb @ W1[:E] + b1)
       v[g,r]  = pos[g,r] @ (W_emb @ W1[:E])
    out[g,i]  = relu(max_j (x1[g,i,j] @ W2) + b2)

Device layout ("dup-halves"): SBUF partitions 0:64 carry the h-dim of the
first 512 rows (32 groups), partitions 64:128 the last 512 rows.

Key trick (PE rate law): K=128 matmuls stream 1 col/cycle; K<=64 streams
at half rate. Weights are block-diagonal [ [W2c, 0], [0, W2c] ] with a
64-channel chunk W2c, so one K=128 pass computes 64 channels for BOTH
halves' pairs simultaneously -- 2x the useful PE rate of the K=64 form.

Drain: PSUM units [128, 2048] = (n4, gi32, j16) per (colblock cq, chunk q).
  R-units: DVE reduce_max straight from PSUM (j16 -> 1), then bias+relu.
  A-units: ACT relu+bias drain to bf16 y (jh-major), then batched DVE
  max-trees over runs of consecutive A-units.
x1 prep: GPSIMD (Pool) computes u + (-v) broadcast; DVE applies relu.
"""

import sys

import numpy as np

try:
    import concourse.bass as bass
except ImportError:  # pragma: no cover
    sys.path.insert(0, "/opt/trn_rl_repo")
    import concourse.bass as bass

from concourse import bacc

import ml_dtypes

import concourse.mybir as mybir
from concourse.bass_utils import run_bass_kernel_spmd
from concourse.tile import TileContext

B, G, P, E, H, BOT = 8192, 512, 16, 64, 64, 1024
NCORES = 8
GC = G // NCORES          # 64 groups per core
RC = GC * P               # 1024 rows per core
HALF_ROWS = RC // 2       # 512 rows per half
NCQ = 4                   # column-quads (superblocks of 2048 x1 cols)
NQ = 16                   # 64-channel chunks
UCOLS = 2048              # x1 cols per unit

FP = mybir.dt.float32
BF = mybir.dt.bfloat16

# per-(cq,q) drain type: 'A' = ACT drain + DVE tree, 'R' = DVE reduce_max
# [A,A,A,R] x4 per cq; in the LAST cq the final R moves to q14 so the
# kernel ends on a single A whose L=1 tree is the only post-drain DVE
# work (the L=2 tree and final reduce hide under the last ACT drain).
UNIT_TYPES = [
    ('R' if (q in (3, 7, 11, 14) if cq == NCQ - 1 else q % 4 == 3) else 'A')
    for cq in range(NCQ) for q in range(NQ)
]
WARMUP = 10  # filler matmuls at start to establish PE p-state

_CACHE = {}


def build_nc():
    nc = bacc.Bacc("TRN2", target_bir_lowering=False, debug=False, num_devices=NCORES)
    phT_d = nc.declare_dram_parameter("phT", [H + 2, RC], BF, isOutput=False)
    AW1_d = nc.declare_dram_parameter("AW1", [H + 2, H], BF, isOutput=False)
    An_d = nc.declare_dram_parameter("Aneg", [2, H], BF, isOutput=False)
    c0_d = nc.declare_dram_parameter("c0d", [128, 1], FP, isOutput=False)
    Wq_d = nc.declare_dram_parameter("Wqd", [128, NQ * 128], BF, isOutput=False)
    b2_d = nc.declare_dram_parameter("b2blk", [128, NQ], FP, isOutput=False)
    out_d = nc.declare_dram_parameter("out", [128, NCQ * NQ * 128], BF, isOutput=True)

    with TileContext(nc) as tc:
        with (
            tc.tile_pool(name="const", bufs=1) as constp,
            tc.tile_pool(name="x1pool", bufs=1) as x1pool,
            tc.tile_pool(name="ypool", bufs=2) as ypool,
            tc.tile_pool(name="tpool", bufs=2) as tpool,
            tc.tile_pool(name="outp", bufs=2) as outp,
        ):
            # ---- constants / inputs (phT/AW1 first: they gate the prep) ----
            phT = constp.tile([H + 2, RC], BF)
            nc.sync.dma_start(phT, phT_d[:, :])
            AW1_sb = constp.tile([H + 2, H], BF)
            nc.scalar.dma_start(AW1_sb, AW1_d[:, :])
            An_sb = constp.tile([2, H], BF)
            nc.scalar.dma_start(An_sb, An_d[:, :])
            c0_sb = constp.tile([128, 1], FP)
            nc.scalar.dma_start(c0_sb, c0_d[:, :])
            Wq_sb = constp.tile([128, NQ * 128], BF)
            nc.gpsimd.dma_start(Wq_sb, Wq_d[:, :])
            b2_sb = constp.tile([128, NQ], FP)
            nc.gpsimd.dma_start(b2_sb, b2_d[:, :])

            uT = constp.tile([128, HALF_ROWS], BF)
            vnT = constp.tile([128, HALF_ROWS], BF)

            # ---- u / -v prep ----
            with tc.tile_pool(name="prepps", bufs=1, space="PSUM") as prepps:
                psum_u = prepps.tile([128, HALF_ROWS], FP)
                psum_v = prepps.tile([128, HALF_ROWS], FP)
                for hh in range(2):
                    usl = psum_u[64 * hh: 64 * (hh + 1), :]
                    vsl = psum_v[64 * hh: 64 * (hh + 1), :]
                    tp = (0, 64 * hh)
                    rs = slice(hh * HALF_ROWS, (hh + 1) * HALF_ROWS)
                    nc.tensor.matmul(vsl, An_sb, phT[0:2, rs],
                                     start=True, stop=True, tile_position=tp)
                    nc.tensor.matmul(usl, AW1_sb, phT[:, rs],
                                     start=True, stop=True, tile_position=tp)
                nc.scalar.add(uT, psum_u, c0_sb)
                nc.vector.tensor_copy(vnT, psum_v)

            # ---- x1 prep: Pool does u + (-v) broadcast, ACT applies relu ----
            # (DVE 2-port modes contend with the Pool engine's SBUF port, so
            # the relu goes to ACT instead of a DVE tensor_scalar.)
            # Subs are all emitted up front (Pool runs them concurrently with
            # the main loop); relus for sbp>=1 are emitted lazily inside the
            # main loop at the R-unit ACT bubbles.
            x1p = x1pool.tile([128, NCQ * UCOLS], BF)
            x1 = x1pool.tile([128, NCQ * UCOLS], BF)
            # sbp0's sub split finely so each relu piece can start as soon
            # as its columns land (the serial Pool chain gates the head)
            sub_parts = [(0, 0, 2), (0, 2, 4), (0, 4, 6), (0, 6, 8)] + [
                (s, 0, 8) for s in range(1, NCQ)]
            for s, g0, g1 in sub_parts:
                u3 = uT[:, s * 128 + g0 * 16: s * 128 + g1 * 16].rearrange(
                    "p (g t) -> p g t", t=P)
                v3 = vnT[:, s * 128 + g0 * 16: s * 128 + g1 * 16].rearrange(
                    "p (g t) -> p g t", t=P)
                u4 = u3.unsqueeze(2).broadcast_to([128, g1 - g0, P, P])
                v4 = v3.unsqueeze(3).broadcast_to([128, g1 - g0, P, P])
                cs = slice(s * UCOLS + g0 * 256, s * UCOLS + g1 * 256)
                x1p4 = x1p[:, cs].rearrange("p (g i j) -> p g i j", i=P, j=P)
                nc.gpsimd.tensor_tensor(x1p4, u4, v4, op=mybir.AluOpType.add)
            # sbp0 relu at head, pieces aligned with the sub pieces
            for c0r, c1r in ((0, 512), (512, 1024), (1024, 1536),
                             (1536, 2048)):
                nc.scalar.activation(
                    x1[:, c0r:c1r], x1p[:, c0r:c1r],
                    mybir.ActivationFunctionType.Relu, bias=0.0, scale=1.0)

            # ---- main loop ----
            # out cols: (cq, q, m) with m = (n4, gi32) = local row index
            with tc.tile_pool(name="psz", bufs=2, space="PSUM") as psz:
                # warmup fillers: establish PE p-state while prep runs
                wtile = psz.tile([128, UCOLS], FP, tag="z", name="warm")
                for r in range(WARMUP):
                    nc.tensor.matmul(
                        wtile[:, (r % 4) * 512:(r % 4 + 1) * 512],
                        Wq_sb[:, 0:128], Wq_sb[:, 0:512],
                        start=True, stop=True,
                    )

                ui = 0
                pending = None  # (pooled, run_start, L, run_y) awaiting tree

                def emit_tree():
                    nonlocal pending
                    if pending is None:
                        return
                    pool_t, run_start, L, run_y = pending
                    pending = None
                    yb = run_y[:, : L * UCOLS]
                    # y layout per unit: (jh2, n4, gi32, j8)
                    y5 = yb.rearrange("p (u jh m j) -> p u jh m j",
                                      u=L, jh=2, j=8)
                    t1 = tpool.tile([128, 4 * 1024], BF, tag="t1", name="t1")
                    t2 = tpool.tile([128, 4 * 512], BF, tag="t2", name="t2")
                    t3 = tpool.tile([128, 4 * 256], BF, tag="t3", name="t3")
                    m1 = t1[:, : L * 1024].rearrange("p (u m j) -> p u m j",
                                                     u=L, j=8)
                    nc.vector.tensor_tensor(
                        m1, y5[:, :, 0], y5[:, :, 1], op=mybir.AluOpType.max)
                    m2 = t2[:, : L * 512].rearrange("p (u m j) -> p u m j",
                                                    u=L, j=4)
                    nc.vector.tensor_tensor(
                        m2, m1[:, :, :, 0:4], m1[:, :, :, 4:8],
                        op=mybir.AluOpType.max)
                    m3 = t3[:, : L * 256].rearrange("p (u m j) -> p u m j",
                                                    u=L, j=2)
                    nc.vector.tensor_tensor(
                        m3, m2[:, :, :, 0:2], m2[:, :, :, 2:4],
                        op=mybir.AluOpType.max)
                    m3f = t3[:, : L * 256].rearrange("p (m two) -> p m two",
                                                     two=2)
                    nc.vector.tensor_tensor(
                        pool_t[:, run_start * 128:(run_start + L) * 128],
                        m3f[:, :, 0], m3f[:, :, 1], op=mybir.AluOpType.max)

                for cq in range(NCQ):
                    pooled = outp.tile([128, NQ * 128], BF, tag="pooled",
                                       name="pooled", bufs=2)
                    run = []
                    run_y = None
                    run_start = -1
                    for q in range(NQ):
                        utype = UNIT_TYPES[ui]
                        ui += 1
                        zt = psz.tile([128, UCOLS], FP, tag="z", name="zt")
                        for n in range(4):
                            nc.tensor.matmul(
                                zt[:, n * 512:(n + 1) * 512],
                                Wq_sb[:, q * 128:(q + 1) * 128],
                                x1[:, cq * UCOLS + n * 512: cq * UCOLS + (n + 1) * 512],
                                start=True, stop=True,
                            )
                        if utype == 'R':
                            rtmp = tpool.tile([128, 128], BF, tag="rt", name="rt",
                                              bufs=3)
                            last = (cq == NCQ - 1 and q == NQ - 1)
                            if run:
                                pending = (pooled, run_start, len(run), run_y)
                                run, run_y, run_start = [], None, -1
                            if last:
                                # tail: tree first so it doesn't serialize
                                # behind the final reduce on the DVE queue
                                emit_tree()
                            nc.vector.reduce_max(
                                rtmp, zt.rearrange("p (m j) -> p m j", j=16),
                                axis=mybir.AxisListType.X)
                            if not last:
                                emit_tree()
                            nc.vector.tensor_scalar(
                                pooled[:, q * 128:(q + 1) * 128], rtmp,
                                b2_sb[:, q:q + 1], 0.0,
                                op0=mybir.AluOpType.add, op1=mybir.AluOpType.max)
                            # lazy x1 relu for the NEXT cq, absorbed into the
                            # ACT bubble at this R-unit position
                            if cq + 1 < NCQ:
                                rp = (cq + 1) * UCOLS + (q // 4) * 512
                                nc.scalar.activation(
                                    x1[:, rp:rp + 512], x1p[:, rp:rp + 512],
                                    mybir.ActivationFunctionType.Relu,
                                    bias=0.0, scale=1.0)
                        else:
                            if not run:
                                run_start = q
                                run_y = ypool.tile([128, 4 * UCOLS], BF, tag="y",
                                                   name="y")
                            pos = len(run)
                            # write y as (jh, n, gi, j8) from psum (n, gi, jh, j8)
                            ysl = run_y[:, pos * UCOLS:(pos + 1) * UCOLS]
                            yv = ysl.rearrange(
                                "p (jh n m j) -> p n m jh j", jh=2, n=4, j=8)
                            zv = zt.rearrange(
                                "p (n m jh j) -> p n m jh j", n=4, jh=2, j=8)
                            if cq == NCQ - 1 and q == NQ - 1:
                                # final unit: drain in n-halves; each half's
                                # tree hides under the other half's drain
                                y5f = ysl.rearrange(
                                    "p (jh m j) -> p jh m j", jh=2, j=8)
                                for h2 in range(2):
                                    ns = slice(2 * h2, 2 * h2 + 2)
                                    nc.scalar.activation(
                                        yv[:, ns], zv[:, ns],
                                        mybir.ActivationFunctionType.Relu,
                                        bias=b2_sb[:, q:q + 1], scale=1.0)
                                    ms = slice(64 * h2, 64 * (h2 + 1))
                                    th1 = tpool.tile([128, 512], BF, tag="th1",
                                                     name="th1", bufs=2)
                                    th2 = tpool.tile([128, 256], BF, tag="th2",
                                                     name="th2", bufs=2)
                                    th3 = tpool.tile([128, 128], BF, tag="th3",
                                                     name="th3", bufs=2)
                                    m1h = th1.rearrange("p (m j) -> p m j", j=8)
                                    nc.vector.tensor_tensor(
                                        m1h, y5f[:, 0, ms], y5f[:, 1, ms],
                                        op=mybir.AluOpType.max)
                                    m2h = th2.rearrange("p (m j) -> p m j", j=4)
                                    nc.vector.tensor_tensor(
                                        m2h, m1h[:, :, 0:4], m1h[:, :, 4:8],
                                        op=mybir.AluOpType.max)
                                    m3h = th3.rearrange("p (m j) -> p m j", j=2)
                                    nc.vector.tensor_tensor(
                                        m3h, m2h[:, :, 0:2], m2h[:, :, 2:4],
                                        op=mybir.AluOpType.max)
                                    m3hf = th3.rearrange(
                                        "p (m two) -> p m two", two=2)
                                    nc.vector.tensor_tensor(
                                        pooled[:, q * 128 + 64 * h2:
                                               q * 128 + 64 * (h2 + 1)],
                                        m3hf[:, :, 0], m3hf[:, :, 1],
                                        op=mybir.AluOpType.max)
                                run_y, run_start = None, -1
                                continue
                            nc.scalar.activation(
                                yv, zv, mybir.ActivationFunctionType.Relu,
                                bias=b2_sb[:, q:q + 1], scale=1.0)
                            run.append(q)
                        if q == 11:
                            dmae = (nc.sync, nc.gpsimd)[cq % 2]
                            dmae.dma_start(
                                out_d[:, cq * NQ * 128: cq * NQ * 128 + 1024],
                                pooled[:, 0:1024])
                        if cq == NCQ - 1 and q == 14:
                            nc.sync.dma_start(
                                out_d[:, cq * NQ * 128 + 1024:
                                      cq * NQ * 128 + 1920],
                                pooled[:, 1024:1920])
                    if run:
                        pending = (pooled, run_start, len(run), run_y)
                        run, run_y, run_start = [], None, -1
                        emit_tree()
                    dmae = (nc.sync, nc.gpsimd)[cq % 2]
                    if cq == NCQ - 1:
                        dmae.dma_start(
                            out_d[:, cq * NQ * 128 + 1920:(cq + 1) * NQ * 128],
                            pooled[:, 1920:2048])
                    else:
                        dmae.dma_start(
                            out_d[:, cq * NQ * 128 + 1024:(cq + 1) * NQ * 128],
                            pooled[:, 1024:2048])
                emit_tree()
    nc.finalize()
    return nc


def _get_nc():
    if "nc" not in _CACHE:
        _CACHE["nc"] = build_nc()
    return _CACHE["nc"]


def kernel(
    in_xy, in_dxdy, h_states, seq_start_end, W_emb, b_emb, W1, b1, W2, b2
):
    pos = np.asarray(in_xy, dtype=np.float32)[-1]  # (B, 2)
    hs = np.asarray(h_states, dtype=np.float32).reshape(B, H)
    W_emb = np.asarray(W_emb, dtype=np.float32)
    b_emb = np.asarray(b_emb, dtype=np.float32)
    W1 = np.asarray(W1, dtype=np.float32)
    b1 = np.asarray(b1, dtype=np.float32)
    W2 = np.asarray(W2, dtype=np.float32)
    b2 = np.asarray(b2, dtype=np.float32)

    A = np.ascontiguousarray(W_emb @ W1[:E])        # (2, H)
    AW1 = np.ascontiguousarray(np.concatenate([A, W1[E:]], axis=0))  # (H+2, H)
    c0 = b_emb @ W1[:E] + b1                        # (H,)
    c0d = np.ascontiguousarray(np.concatenate([c0, c0])[:, None])  # (128,1)
    # block-diagonal weight tiles: Wq[:, q*128:(q+1)*128] =
    #   [[W2[:, 64q:64q+64], 0], [0, W2[:, 64q:64q+64]]]
    Wqd = np.zeros((128, NQ * 128), dtype=np.float32)
    for q in range(NQ):
        blk = W2[:, 64 * q: 64 * (q + 1)]           # (64, 64)
        Wqd[0:64, q * 128: q * 128 + 64] = blk
        Wqd[64:128, q * 128 + 64: q * 128 + 128] = blk
    # bias per (partition, q): channel = q*64 + (p % 64)
    b2blk = np.ascontiguousarray(
        np.concatenate([b2.reshape(NQ, 64).T, b2.reshape(NQ, 64).T], axis=0)
    ).astype(np.float32)                            # (128, NQ)

    in_maps = []
    for cid in range(NCORES):
        rs = slice(cid * RC, (cid + 1) * RC)
        in_maps.append(
            {
                "phT": np.ascontiguousarray(
                    np.concatenate([pos[rs], hs[rs]], axis=1).T
                ).astype(ml_dtypes.bfloat16),
                "AW1": AW1.astype(ml_dtypes.bfloat16),
                "Aneg": (-A).astype(ml_dtypes.bfloat16),
                "c0d": c0d,
                "Wqd": Wqd.astype(ml_dtypes.bfloat16),
                "b2blk": b2blk,
            }
        )

    _CACHE["in_maps"] = in_maps
    nc = _get_nc()
    res = run_bass_kernel_spmd(nc, in_maps, core_ids=list(range(NCORES)))
    # out[p, (cq, q, m)]: row = (p//64)*512 + cq*128 + m; ch = q*64 + p%64
    outs = []
    for r in res.results:
        a = np.asarray(r["out"], dtype=np.float32)      # (128, 8192)
        a = a.reshape(2, 64, NCQ, NQ, 128)              # (half, chin, cq, q, m)
        a = a.transpose(0, 2, 4, 3, 1)                  # (half, cq, m, q, chin)
        outs.append(a.reshape(RC, BOT))
    return np.concatenate(outs, axis=0)


if __name__ == "__main__":
    rng = np.random.default_rng(0)
    inputs = {
        "in_xy": rng.standard_normal((8, B, 2), dtype=np.float32),
        "in_dxdy": rng.standard_normal((8, B, 2), dtype=np.float32),
        "h_states": rng.standard_normal((1, B, H), dtype=np.float32),
        "seq_start_end": np.stack(
            [np.arange(G) * P, np.arange(G) * P + P], axis=1
        ).astype(np.int64),
        "W_emb": rng.standard_normal((2, E), dtype=np.float32),
        "b_emb": np.zeros(E, dtype=np.float32),
        "W1": rng.standard_normal((E + H, H), dtype=np.float32),
        "b1": np.zeros(H, dtype=np.float32),
        "W2": rng.standard_normal((H, BOT), dtype=np.float32),
        "b2": np.zeros(BOT, dtype=np.float32),
    }
    out = kernel(**inputs)
    print(out.shape, out.dtype)


# revision 29
# speedup vs baseline: 1.0017x; 1.0017x over previous
"""Trainium2 Bass kernel for nn_PoolHiddenNet (gnn_message_passing).

Reference computation (uniform contiguous groups of P=16):
    pos = in_xy[-1]                       # (B, 2)
    rel[g,i,j] = pos[g,j] - pos[g,i]
    emb = rel @ W_emb + b_emb             # (G,P,P,E)
    x   = concat([emb, h[g,j]], -1)
    x1  = relu(x @ W1 + b1)               # (G,P,P,H)
    x2  = relu(x1 @ W2 + b2)              # (G,P,P,BOT)
    out = max over j -> (B, BOT)

Algebraic restructuring:
    x1[g,i,j] = relu(u[g,j] - v[g,i])
       u[g,r]  = pos[g,r] @ (W_emb @ W1[:E]) + h[g,r] @ W1[E:] + (b_emb @ W1[:E] + b1)
       v[g,r]  = pos[g,r] @ (W_emb @ W1[:E])
    out[g,i]  = relu(max_j (x1[g,i,j] @ W2) + b2)

Device layout ("dup-halves"): SBUF partitions 0:64 carry the h-dim of the
first 512 rows (32 groups), partitions 64:128 the last 512 rows.

Key trick (PE rate law): K=128 matmuls stream 1 col/cycle; K<=64 streams
at half rate. Weights are block-diagonal [ [W2c, 0], [0, W2c] ] with a
64-channel chunk W2c, so one K=128 pass computes 64 channels for BOTH
halves' pairs simultaneously -- 2x the useful PE rate of the K=64 form.

Drain: PSUM units [128, 2048] = (n4, gi32, j16) per (colblock cq, chunk q).
  R-units: DVE reduce_max straight from PSUM (j16 -> 1), then bias+relu.
  A-units: ACT relu+bias drain to bf16 y (jh-major), then batched DVE
  max-trees over runs of consecutive A-units.
x1 prep: GPSIMD (Pool) computes u + (-v) broadcast; DVE applies relu.
"""

import sys

import numpy as np

try:
    import concourse.bass as bass
except ImportError:  # pragma: no cover
    sys.path.insert(0, "/opt/trn_rl_repo")
    import concourse.bass as bass

from concourse import bacc

import ml_dtypes

import concourse.mybir as mybir
from concourse.bass_utils import run_bass_kernel_spmd
from concourse.tile import TileContext

B, G, P, E, H, BOT = 8192, 512, 16, 64, 64, 1024
NCORES = 8
GC = G // NCORES          # 64 groups per core
RC = GC * P               # 1024 rows per core
HALF_ROWS = RC // 2       # 512 rows per half
NCQ = 4                   # column-quads (superblocks of 2048 x1 cols)
NQ = 16                   # 64-channel chunks
UCOLS = 2048              # x1 cols per unit

FP = mybir.dt.float32
BF = mybir.dt.bfloat16

# per-(cq,q) drain type: 'A' = ACT drain + DVE tree, 'R' = DVE reduce_max
# [A,A,A,R] x4 per cq; in the LAST cq the final R moves to q14 so the
# kernel ends on a single A whose L=1 tree is the only post-drain DVE
# work (the L=2 tree and final reduce hide under the last ACT drain).
UNIT_TYPES = [
    ('R' if (q in (3, 7, 11, 14) if cq == NCQ - 1 else q % 4 == 3) else 'A')
    for cq in range(NCQ) for q in range(NQ)
]
WARMUP = 10  # filler matmuls at start to establish PE p-state

_CACHE = {}


def build_nc():
    nc = bacc.Bacc("TRN2", target_bir_lowering=False, debug=False, num_devices=NCORES)
    phT_d = nc.declare_dram_parameter("phT", [H + 2, RC], BF, isOutput=False)
    AW1_d = nc.declare_dram_parameter("AW1", [H + 2, H], BF, isOutput=False)
    An_d = nc.declare_dram_parameter("Aneg", [2, H], BF, isOutput=False)
    c0_d = nc.declare_dram_parameter("c0d", [128, 1], FP, isOutput=False)
    Wq_d = nc.declare_dram_parameter("Wqd", [128, NQ * 128], BF, isOutput=False)
    b2_d = nc.declare_dram_parameter("b2blk", [128, NQ], FP, isOutput=False)
    out_d = nc.declare_dram_parameter("out", [128, NCQ * NQ * 128], BF, isOutput=True)

    with TileContext(nc) as tc:
        with (
            tc.tile_pool(name="const", bufs=1) as constp,
            tc.tile_pool(name="x1pool", bufs=1) as x1pool,
            tc.tile_pool(name="ypool", bufs=2) as ypool,
            tc.tile_pool(name="tpool", bufs=2) as tpool,
            tc.tile_pool(name="outp", bufs=2) as outp,
        ):
            # ---- constants / inputs (phT/AW1 first: they gate the prep) ----
            phT = constp.tile([H + 2, RC], BF)
            nc.sync.dma_start(phT, phT_d[:, :])
            AW1_sb = constp.tile([H + 2, H], BF)
            nc.scalar.dma_start(AW1_sb, AW1_d[:, :])
            An_sb = constp.tile([2, H], BF)
            nc.scalar.dma_start(An_sb, An_d[:, :])
            c0_sb = constp.tile([128, 1], FP)
            nc.scalar.dma_start(c0_sb, c0_d[:, :])
            Wq_sb = constp.tile([128, NQ * 128], BF)
            nc.gpsimd.dma_start(Wq_sb, Wq_d[:, :])
            b2_sb = constp.tile([128, NQ], FP)
            nc.gpsimd.dma_start(b2_sb, b2_d[:, :])

            uT = constp.tile([128, HALF_ROWS], BF)
            vnT = constp.tile([128, HALF_ROWS], BF)

            # ---- u / -v prep ----
            with tc.tile_pool(name="prepps", bufs=1, space="PSUM") as prepps:
                psum_u = prepps.tile([128, HALF_ROWS], FP)
                psum_v = prepps.tile([128, HALF_ROWS], FP)
                for hh in range(2):
                    usl = psum_u[64 * hh: 64 * (hh + 1), :]
                    vsl = psum_v[64 * hh: 64 * (hh + 1), :]
                    tp = (0, 64 * hh)
                    rs = slice(hh * HALF_ROWS, (hh + 1) * HALF_ROWS)
                    nc.tensor.matmul(vsl, An_sb, phT[0:2, rs],
                                     start=True, stop=True, tile_position=tp)
                    nc.tensor.matmul(usl, AW1_sb, phT[:, rs],
                                     start=True, stop=True, tile_position=tp)
                nc.scalar.add(uT, psum_u, c0_sb)
                nc.vector.tensor_copy(vnT, psum_v)

            # ---- x1 prep: Pool does u + (-v) broadcast, ACT applies relu ----
            # (DVE 2-port modes contend with the Pool engine's SBUF port, so
            # the relu goes to ACT instead of a DVE tensor_scalar.)
            # Subs are all emitted up front (Pool runs them concurrently with
            # the main loop); relus for sbp>=1 are emitted lazily inside the
            # main loop at the R-unit ACT bubbles.
            x1p = x1pool.tile([128, NCQ * UCOLS], BF)
            x1 = x1pool.tile([128, NCQ * UCOLS], BF)
            # sbp0's sub split in half so the first relu can start sooner
            sub_parts = [(0, 0, 2), (0, 2, 4), (0, 4, 8)] + [
                (s, 0, 8) for s in range(1, NCQ)]
            for s, g0, g1 in sub_parts:
                u3 = uT[:, s * 128 + g0 * 16: s * 128 + g1 * 16].rearrange(
                    "p (g t) -> p g t", t=P)
                v3 = vnT[:, s * 128 + g0 * 16: s * 128 + g1 * 16].rearrange(
                    "p (g t) -> p g t", t=P)
                u4 = u3.unsqueeze(2).broadcast_to([128, g1 - g0, P, P])
                v4 = v3.unsqueeze(3).broadcast_to([128, g1 - g0, P, P])
                cs = slice(s * UCOLS + g0 * 256, s * UCOLS + g1 * 256)
                x1p4 = x1p[:, cs].rearrange("p (g i j) -> p g i j", i=P, j=P)
                nc.gpsimd.tensor_tensor(x1p4, u4, v4, op=mybir.AluOpType.add)
            # sbp0 relu at head, split for latency
            for c0r, c1r in ((0, 512), (512, 1024), (1024, 2048)):
                nc.scalar.activation(
                    x1[:, c0r:c1r], x1p[:, c0r:c1r],
                    mybir.ActivationFunctionType.Relu, bias=0.0, scale=1.0)

            # ---- main loop ----
            # out cols: (cq, q, m) with m = (n4, gi32) = local row index
            with tc.tile_pool(name="psz", bufs=2, space="PSUM") as psz:
                # warmup fillers: establish PE p-state while prep runs
                wtile = psz.tile([128, UCOLS], FP, tag="z", name="warm")
                for r in range(WARMUP):
                    nc.tensor.matmul(
                        wtile[:, (r % 4) * 512:(r % 4 + 1) * 512],
                        Wq_sb[:, 0:128], Wq_sb[:, 0:512],
                        start=True, stop=True,
                    )

                ui = 0
                pending = None  # (pooled, run_start, L, run_y) awaiting tree

                def emit_tree():
                    nonlocal pending
                    if pending is None:
                        return
                    pool_t, run_start, L, run_y = pending
                    pending = None
                    yb = run_y[:, : L * UCOLS]
                    # y layout per unit: (jh2, n4, gi32, j8)
                    y5 = yb.rearrange("p (u jh m j) -> p u jh m j",
                                      u=L, jh=2, j=8)
                    t1 = tpool.tile([128, 4 * 1024], BF, tag="t1", name="t1")
                    t2 = tpool.tile([128, 4 * 512], BF, tag="t2", name="t2")
                    t3 = tpool.tile([128, 4 * 256], BF, tag="t3", name="t3")
                    m1 = t1[:, : L * 1024].rearrange("p (u m j) -> p u m j",
                                                     u=L, j=8)
                    nc.vector.tensor_tensor(
                        m1, y5[:, :, 0], y5[:, :, 1], op=mybir.AluOpType.max)
                    m2 = t2[:, : L * 512].rearrange("p (u m j) -> p u m j",
                                                    u=L, j=4)
                    nc.vector.tensor_tensor(
                        m2, m1[:, :, :, 0:4], m1[:, :, :, 4:8],
                        op=mybir.AluOpType.max)
                    m3 = t3[:, : L * 256].rearrange("p (u m j) -> p u m j",
                                                    u=L, j=2)
                    nc.vector.tensor_tensor(
                        m3, m2[:, :, :, 0:2], m2[:, :, :, 2:4],
                        op=mybir.AluOpType.max)
                    m3f = t3[:, : L * 256].rearrange("p (m two) -> p m two",
                                                     two=2)
                    nc.vector.tensor_tensor(
                        pool_t[:, run_start * 128:(run_start + L) * 128],
                        m3f[:, :, 0], m3f[:, :, 1], op=mybir.AluOpType.max)

                for cq in range(NCQ):
                    pooled = outp.tile([128, NQ * 128], BF, tag="pooled",
                                       name="pooled", bufs=2)
                    run = []
                    run_y = None
                    run_start = -1
                    for q in range(NQ):
                        utype = UNIT_TYPES[ui]
                        ui += 1
                        zt = psz.tile([128, UCOLS], FP, tag="z", name="zt")
                        for n in range(4):
                            nc.tensor.matmul(
                                zt[:, n * 512:(n + 1) * 512],
                                Wq_sb[:, q * 128:(q + 1) * 128],
                                x1[:, cq * UCOLS + n * 512: cq * UCOLS + (n + 1) * 512],
                                start=True, stop=True,
                            )
                        if utype == 'R':
                            rtmp = tpool.tile([128, 128], BF, tag="rt", name="rt",
                                              bufs=3)
                            # at the last cq's final R (q14) nothing waits on
                            # this unit's PSUM banks, so emit the pending tree
                            # FIRST: its inputs are ready ~2us before the
                            # reduce's fill, shortening the DVE tail chain
                            tree_first = (cq == NCQ - 1 and q >= NQ - 2)
                            if run:
                                pending = (pooled, run_start, len(run), run_y)
                                run, run_y, run_start = [], None, -1
                            if tree_first:
                                emit_tree()
                            nc.vector.reduce_max(
                                rtmp, zt.rearrange("p (m j) -> p m j", j=16),
                                axis=mybir.AxisListType.X)
                            if not tree_first:
                                emit_tree()
                            nc.vector.tensor_scalar(
                                pooled[:, q * 128:(q + 1) * 128], rtmp,
                                b2_sb[:, q:q + 1], 0.0,
                                op0=mybir.AluOpType.add, op1=mybir.AluOpType.max)
                            # lazy x1 relu for the NEXT cq, absorbed into the
                            # ACT bubble at this R-unit position
                            if cq + 1 < NCQ:
                                rp = (cq + 1) * UCOLS + (q // 4) * 512
                                nc.scalar.activation(
                                    x1[:, rp:rp + 512], x1p[:, rp:rp + 512],
                                    mybir.ActivationFunctionType.Relu,
                                    bias=0.0, scale=1.0)
                        else:
                            if not run:
                                run_start = q
                                run_y = ypool.tile([128, 4 * UCOLS], BF, tag="y",
                                                   name="y")
                            pos = len(run)
                            # write y as (jh, n, gi, j8) from psum (n, gi, jh, j8)
                            ysl = run_y[:, pos * UCOLS:(pos + 1) * UCOLS]
                            yv = ysl.rearrange(
                                "p (jh n m j) -> p n m jh j", jh=2, n=4, j=8)
                            zv = zt.rearrange(
                                "p (n m jh j) -> p n m jh j", n=4, jh=2, j=8)
                            if cq == NCQ - 1 and q == NQ - 1:
                                # final unit: drain in n-halves; each half's
                                # tree hides under the other half's drain
                                y5f = ysl.rearrange(
                                    "p (jh m j) -> p jh m j", jh=2, j=8)
                                for h2 in range(2):
                                    ns = slice(2 * h2, 2 * h2 + 2)
                                    nc.scalar.activation(
                                        yv[:, ns], zv[:, ns],
                                        mybir.ActivationFunctionType.Relu,
                                        bias=b2_sb[:, q:q + 1], scale=1.0)
                                    ms = slice(64 * h2, 64 * (h2 + 1))
                                    th1 = tpool.tile([128, 512], BF, tag="th1",
                                                     name="th1", bufs=2)
                                    th2 = tpool.tile([128, 256], BF, tag="th2",
                                                     name="th2", bufs=2)
                                    th3 = tpool.tile([128, 128], BF, tag="th3",
                                                     name="th3", bufs=2)
                                    m1h = th1.rearrange("p (m j) -> p m j", j=8)
                                    nc.vector.tensor_tensor(
                                        m1h, y5f[:, 0, ms], y5f[:, 1, ms],
                                        op=mybir.AluOpType.max)
                                    m2h = th2.rearrange("p (m j) -> p m j", j=4)
                                    nc.vector.tensor_tensor(
                                        m2h, m1h[:, :, 0:4], m1h[:, :, 4:8],
                                        op=mybir.AluOpType.max)
                                    m3h = th3.rearrange("p (m j) -> p m j", j=2)
                                    nc.vector.tensor_tensor(
                                        m3h, m2h[:, :, 0:2], m2h[:, :, 2:4],
                                        op=mybir.AluOpType.max)
                                    m3hf = th3.rearrange(
                                        "p (m two) -> p m two", two=2)
                                    nc.vector.tensor_tensor(
                                        pooled[:, q * 128 + 64 * h2:
                                               q * 128 + 64 * (h2 + 1)],
                                        m3hf[:, :, 0], m3hf[:, :, 1],
                                        op=mybir.AluOpType.max)
                                run_y, run_start = None, -1
                                continue
                            nc.scalar.activation(
                                yv, zv, mybir.ActivationFunctionType.Relu,
                                bias=b2_sb[:, q:q + 1], scale=1.0)
                            run.append(q)
                        if q == 11:
                            dmae = (nc.sync, nc.gpsimd)[cq % 2]
                            dmae.dma_start(
                                out_d[:, cq * NQ * 128: cq * NQ * 128 + 1024],
                                pooled[:, 0:1024])
                        if cq == NCQ - 1 and q == 14:
                            nc.sync.dma_start(
                                out_d[:, cq * NQ * 128 + 1024:
                                      cq * NQ * 128 + 1920],
                                pooled[:, 1024:1920])
                    if run:
                        pending = (pooled, run_start, len(run), run_y)
                        run, run_y, run_start = [], None, -1
                        emit_tree()
                    dmae = (nc.sync, nc.gpsimd)[cq % 2]
                    if cq == NCQ - 1:
                        dmae.dma_start(
                            out_d[:, cq * NQ * 128 + 1920:(cq + 1) * NQ * 128],
                            pooled[:, 1920:2048])
                    else:
                        dmae.dma_start(
                            out_d[:, cq * NQ * 128 + 1024:(cq + 1) * NQ * 128],
                            pooled[:, 1024:2048])
                emit_tree()
    nc.finalize()
    return nc


def _get_nc():
    if "nc" not in _CACHE:
        _CACHE["nc"] = build_nc()
    return _CACHE["nc"]


def kernel(
    in_xy, in_dxdy, h_states, seq_start_end, W_emb, b_emb, W1, b1, W2, b2
):
    pos = np.asarray(in_xy, dtype=np.float32)[-1]  # (B, 2)
    hs = np.asarray(h_states, dtype=np.float32).reshape(B, H)
    W_emb = np.asarray(W_emb, dtype=np.float32)
    b_emb = np.asarray(b_emb, dtype=np.float32)
    W1 = np.asarray(W1, dtype=np.float32)
    b1 = np.asarray(b1, dtype=np.float32)
    W2 = np.asarray(W2, dtype=np.float32)
    b2 = np.asarray(b2, dtype=np.float32)

    A = np.ascontiguousarray(W_emb @ W1[:E])        # (2, H)
    AW1 = np.ascontiguousarray(np.concatenate([A, W1[E:]], axis=0))  # (H+2, H)
    c0 = b_emb @ W1[:E] + b1                        # (H,)
    c0d = np.ascontiguousarray(np.concatenate([c0, c0])[:, None])  # (128,1)
    # block-diagonal weight tiles: Wq[:, q*128:(q+1)*128] =
    #   [[W2[:, 64q:64q+64], 0], [0, W2[:, 64q:64q+64]]]
    Wqd = np.zeros((128, NQ * 128), dtype=np.float32)
    for q in range(NQ):
        blk = W2[:, 64 * q: 64 * (q + 1)]           # (64, 64)
        Wqd[0:64, q * 128: q * 128 + 64] = blk
        Wqd[64:128, q * 128 + 64: q * 128 + 128] = blk
    # bias per (partition, q): channel = q*64 + (p % 64)
    b2blk = np.ascontiguousarray(
        np.concatenate([b2.reshape(NQ, 64).T, b2.reshape(NQ, 64).T], axis=0)
    ).astype(np.float32)                            # (128, NQ)

    in_maps = []
    for cid in range(NCORES):
        rs = slice(cid * RC, (cid + 1) * RC)
        in_maps.append(
            {
                "phT": np.ascontiguousarray(
                    np.concatenate([pos[rs], hs[rs]], axis=1).T
                ).astype(ml_dtypes.bfloat16),
                "AW1": AW1.astype(ml_dtypes.bfloat16),
                "Aneg": (-A).astype(ml_dtypes.bfloat16),
                "c0d": c0d,
                "Wqd": Wqd.astype(ml_dtypes.bfloat16),
                "b2blk": b2blk,
            }
        )

    _CACHE["in_maps"] = in_maps
    nc = _get_nc()
    res = run_bass_kernel_spmd(nc, in_maps, core_ids=list(range(NCORES)))
    # out[p, (cq, q, m)]: row = (p//64)*512 + cq*128 + m; ch = q*64 + p%64
    outs = []
    for r in res.results:
        a = np.asarray(r["out"], dtype=np.float32)      # (128, 8192)
        a = a.reshape(2, 64, NCQ, NQ, 128)              # (half, chin, cq, q, m)
        a = a.transpose(0, 2, 4, 3, 1)                  # (half, cq, m, q, chin)
        outs.append(a.reshape(RC, BOT))
    return np.concatenate(outs, axis=0)


if __name__ == "__main__":
    rng = np.random.default_rng(0)
    inputs = {
        "in_xy": rng.standard_normal((8, B, 2), dtype=np.float32),
        "in_dxdy": rng.standard_normal((8, B, 2), dtype=np.float32),
        "h_states": rng.standard_normal((1, B, H), dtype=np.float32),
        "seq_start_end": np.stack(
            [np.arange(G) * P, np.arange(G) * P + P], axis=1
        ).astype(np.int64),
        "W_emb": rng.standard_normal((2, E), dtype=np.float32),
        "b_emb": np.zeros(E, dtype=np.float32),
        "W1": rng.standard_normal((E + H, H), dtype=np.float32),
        "b1": np.zeros(H, dtype=np.float32),
        "W2": rng.standard_normal((H, BOT), dtype=np.float32),
        "b2": np.zeros(BOT, dtype=np.float32),
    }
    out = kernel(**inputs)
    print(out.shape, out.dtype)
